# revision 1
# baseline (speedup 1.0000x reference)
"""RAFT correlation-pyramid lookup kernel for 8 trn2 NeuronCores.

Shard: each core takes all 4 batches x 8 pixel rows (i-axis) of the
64x64 grid.  Per core:
  1. matmul (fp16) f1^T @ [f2 | host-pooled f2 pyramids] -> corr volumes
  2. ACT drains PSUM -> SBUF fp16 (2048-wide), DMA to a DRAM bounce tile
  3. element-granular indirect-DMA gathers per-pixel dynamic windows
  4. DVE reconstructs bilinear lookups as separable 3-tap MACs, with all
     operands fp16 / packed-last-dim
Weights / gather indices are computed on-device from the flow input; the
x-axis chain runs on DVE while the y-axis chain runs on GPSIMD so they
overlap.  All tile pools share one scope so no barrier splits the head
from the matmul pipeline.  The diamond extraction happens on the host.
"""

import os
import sys

import numpy as np

sys.path.insert(0, "/opt/trn_rl_repo")

B = 4
C = 256
H8 = W8 = 64
RI = 8  # i-rows per core
NCORES = 8
NT = 16  # pixel tiles per core (128 pixels each)
NPIX = NT * 128

HK = [64, 32, 16, 8]  # level map sizes
SEG = [0, 4096, 5120, 5376]  # level offsets within a pixel's volume
VOLW = 5440  # per-pixel volume length (all 4 levels)
NL = 41  # diamond offsets
ROWLEN = [1, 3, 5, 7, 9, 7, 5, 3, 1]  # diamond row lengths (dy=-4..4)
# diamond -> 9x9 rect grid (layout x*9 + y)
IDX81 = np.array(
    [(abs(d - 4) + c) * 9 + d for d in range(9) for c in range(ROWLEN[d])],
    dtype=np.int64,
)

_CACHE = {}


def _consts(core):
    """Constant input tensors for one core (fp32), compact layout."""
    p = np.arange(128)
    t = np.arange(NT)
    k = np.arange(4)
    j = np.arange(9)
    hk = np.array(HK, dtype=np.float64)

    # i/j pixel coordinates:  t=(b,ihi), part=(i2,jcol)
    ihi = t % 4
    i2 = p // 64
    iconst = (8 * core + ihi[None, :] * 2 + i2[:, None]).astype(np.float32)
    jconst = (p % 64).astype(np.float32)[:, None]

    def rep4(a):  # [4] -> [128, 4]
        return np.broadcast_to(np.asarray(a, np.float64)[None], (128, 4)).astype(np.float32)

    def rep36(a):  # [9,4] -> [128, 36]
        return np.broadcast_to(np.asarray(a, np.float64).reshape(36)[None], (128, 36)).astype(np.float32)

    segk = np.array(SEG, dtype=np.float64)
    cbase4 = (p[:, None] * VOLW + VOLW + segk[None, :] - hk[None, :] - 1.0).astype(np.float32)

    d = {
        "iconst": iconst,
        "jconst": jconst.astype(np.float32),
        "cinv4": rep4(0.5**k),
        "chi4": rep4(hk + 5.2),
        "ct4": rep4((hk - 1.0) / hk),
        "cw4": rep4(hk),
        "cbase4": cbase4,
        "cdyt": rep36((j - 4)[:, None] * ((hk - 1.0) / hk)[None, :]),
        "ch05": rep36(np.broadcast_to((hk - 0.5)[None, :], (9, 4))),
        "ch15": rep36(np.broadcast_to((hk - 1.5)[None, :], (9, 4))),
        "cj": rep36(np.broadcast_to(j[:, None].astype(np.float64), (9, 4))),
        "csk": rep36(np.broadcast_to((0.0625 * 0.25**k)[None, :], (9, 4))),
    }
    return {"cstack": np.concatenate([d[n] for n in CNAMES], axis=1)}


CWID = {
    "iconst": NT, "jconst": 1,
    "cinv4": 4, "chi4": 4, "ct4": 4, "cw4": 4, "cbase4": 4,
    "cdyt": 36, "ch05": 36, "ch15": 36, "cj": 36, "csk": 36,
}
CNAMES = list(CWID)
CTOT = sum(CWID.values())


def _build():
    import concourse.bass as bass
    import concourse.tile as tile
    from concourse import bacc, mybir

    f32 = mybir.dt.float32
    f16 = mybir.dt.float16
    i32 = mybir.dt.int32
    Alu = mybir.AluOpType

    nc = bacc.Bacc("TRN2", target_bir_lowering=False, debug=False, num_devices=NCORES)

    f1 = nc.dram_tensor("f1", [B, C, RI, W8], f16, kind="ExternalInput")
    f2 = nc.dram_tensor("f2", [B, C, H8, W8], f16, kind="ExternalInput")
    pyr = nc.dram_tensor("pyr", [B, C, 1344], f16, kind="ExternalInput")
    flow = nc.dram_tensor("flow", [B, 2, RI, W8], f32, kind="ExternalInput")
    cdram = nc.dram_tensor("cstack", [128, CTOT], f32, kind="ExternalInput")
    out = nc.dram_tensor("out", [NPIX, 324], f16, kind="ExternalOutput")

    with tile.TileContext(nc) as tc:
        with (
            tc.tile_pool(name="const", bufs=1) as cp,
            tc.tile_pool(name="wts", bufs=1) as wp,
            tc.tile_pool(name="wscratch", bufs=1) as sp,
            tc.tile_pool(name="main", bufs=2) as mp,
            tc.tile_pool(name="fio", bufs=4) as fp,
            tc.tile_pool(name="dram", bufs=3, space="DRAM") as dp,
            tc.tile_pool(name="psum", bufs=2, space="PSUM") as pp,
        ):
            # ---- flow -> lookup weights + gather indices ----
            # x-axis chain on DVE, y-axis chain on GPSIMD (runs concurrently)
            W = {}
            idx32 = wp.tile([128, 64], i32, tag="idx32", name="idx32")
            fall = sp.tile([128, 32], f32, tag="fall", name="fall")
            fsrc = flow[:].rearrange("b c (ihi i2) j -> (i2 j) b (c ihi)", i2=2)
            nc.sync.dma_start(out=fall[:], in_=fsrc)
            cstack = cp.tile([128, CTOT], f32, tag="cstack", name="cstack")
            nc.scalar.dma_start(out=cstack[:], in_=cdram[:])
            ct = {}
            off = 0
            for n in CNAMES:
                ct[n] = cstack[:, off:off + CWID[n]]
                off += CWID[n]

            def bc4(name):  # [p,4] const -> (k,t) broadcast view
                return ct[name][:].unsqueeze(2).to_broadcast([128, 4, NT])

            def bc36(name):  # [p,36] const -> (j,k,t) broadcast view
                return ct[name][:].rearrange("p (j k) -> p j k", j=9).unsqueeze(
                    3).to_broadcast([128, 9, 4, NT])

            # Phase 1 (DVE, both axes): comparison-heavy front producing
            # y0f / frac / v0 / v1.  GPSIMD only supports add/sub/mult.
            Y0J0, FRAC, V0, V1 = {}, {}, {}, {}
            for ax, coord_ch in (("y", 0), ("x", 1)):
                eng = nc.vector
                f3 = fall[:].rearrange("p (b c ihi) -> p b c ihi", b=B, c=2)
                coord = sp.tile([128, NT], f32, tag=f"coord_{ax}", name=f"coord{ax}")
                if ax == "y":
                    eng.tensor_tensor(
                        out=coord[:].rearrange("p (b h) -> p b h", b=B),
                        in0=f3[:, :, 0, :],
                        in1=ct["iconst"][:].rearrange("p (b h) -> p b h", b=B), op=Alu.add)
                else:
                    eng.tensor_tensor(
                        out=coord[:].rearrange("p (b h) -> p b h", b=B),
                        in0=f3[:, :, 1, :],
                        in1=ct["jconst"][:].to_broadcast([128, NT]).rearrange(
                            "p (b h) -> p b h", b=B), op=Alu.add)

                ck = sp.tile([128, 64], f32, tag=f"ck_{ax}", name=f"ck{ax}")
                ckv = ck[:].rearrange("p (k t) -> p k t", k=4)
                eng.tensor_tensor(
                    out=ckv, in0=coord[:].unsqueeze(1).to_broadcast([128, 4, NT]),
                    in1=bc4("cinv4"), op=Alu.mult)
                eng.tensor_tensor(out=ckv, in0=ckv, in1=bc4("chi4"), op=Alu.min)
                eng.tensor_scalar_max(ck[:], ck[:], -5.2)
                eng.tensor_tensor(out=ckv, in0=ckv, in1=bc4("ct4"), op=Alu.mult)

                ybar = sp.tile([128, 576], f32, tag=f"ybar_{ax}", name=f"ybar{ax}")
                ybv = ybar[:].rearrange("p (j k t) -> p j k t", j=9, k=4)
                eng.tensor_tensor(
                    out=ybv,
                    in0=ck[:].unsqueeze(1).to_broadcast([128, 9, 64]).rearrange(
                        "p j (k t) -> p j k t", k=4),
                    in1=bc36("cdyt"), op=Alu.add)
                # exact floor: round-to-nearest via +1.5*2^23, then correct
                frac = sp.tile([128, 576], f32, tag=f"frac_{ax}", name=f"frac{ax}")
                y0f = sp.tile([128, 576], f32, tag=f"y0f_{ax}", name=f"y0f{ax}")
                cmp = sp.tile([128, 576], f32, tag=f"cmp_{ax}", name=f"cmp{ax}")
                eng.tensor_scalar(y0f[:], ybar[:], 12582912.0, -12582912.0,
                                  op0=Alu.add, op1=Alu.add)
                eng.tensor_tensor(out=cmp[:], in0=y0f[:], in1=ybar[:], op=Alu.is_gt)
                eng.tensor_sub(y0f[:], y0f[:], cmp[:])
                eng.tensor_sub(frac[:], ybar[:], y0f[:])
                Y0J0[ax], FRAC[ax] = y0f, frac
                y0v = y0f[:].rearrange("p (j k t) -> p j k t", j=9, k=4)

                v0 = sp.tile([128, 576], f32, tag=f"v0_{ax}", name=f"v0{ax}")
                v1 = sp.tile([128, 576], f32, tag=f"v1_{ax}", name=f"v1{ax}")
                tmp = sp.tile([128, 576], f32, tag=f"tmp_{ax}", name=f"tmp{ax}")
                tmpv = tmp[:].rearrange("p (j k t) -> p j k t", j=9, k=4)
                # valid(y0):   y0 in [0, h-1]
                eng.tensor_scalar(v0[:], y0f[:], -0.1, None, op0=Alu.is_ge)
                eng.tensor_tensor(out=tmpv, in0=y0v, in1=bc36("ch05"), op=Alu.is_le)
                eng.tensor_mul(v0[:], v0[:], tmp[:])
                # valid(y0+1): y0 in [-1, h-2]
                eng.tensor_scalar(v1[:], y0f[:], -1.1, None, op0=Alu.is_ge)
                eng.tensor_tensor(out=tmpv, in0=y0v, in1=bc36("ch15"), op=Alu.is_le)
                eng.tensor_mul(v1[:], v1[:], tmp[:])
                V0[ax], V1[ax] = v0, v1

            # Phase 2: bilinear weights + 3-tap combine (DVE; GPSIMD's Q7
            # path produced NaNs on HW with broadcast reads/strided writes).
            for ax in ("y", "x"):
                eng = nc.vector
                y0f, frac, v0, v1 = Y0J0[ax], FRAC[ax], V0[ax], V1[ax]
                w0 = sp.tile([128, 576], f32, tag=f"w0_{ax}", name=f"w0{ax}")
                w1 = sp.tile([128, 576], f32, tag=f"w1_{ax}", name=f"w1{ax}")
                eng.tensor_mul(w1[:], frac[:], v1[:])
                # w0 = (1-frac)*v0 = v0 - frac*v0
                eng.tensor_mul(w0[:], frac[:], v0[:])
                eng.tensor_sub(w0[:], v0[:], w0[:])
                if ax == "x":  # fold level scale (1/16 * 4^-k) into x weights
                    w0v = w0[:].rearrange("p (j k t) -> p j k t", j=9, k=4)
                    w1v = w1[:].rearrange("p (j k t) -> p j k t", j=9, k=4)
                    eng.tensor_tensor(out=w0v, in0=w0v, in1=bc36("csk"), op=Alu.mult)
                    eng.tensor_tensor(out=w1v, in0=w1v, in1=bc36("csk"), op=Alu.mult)

                ey = sp.tile([128, 576], f32, tag=f"ey_{ax}", name=f"ey{ax}")
                eyv = ey[:].rearrange("p (j k t) -> p j k t", j=9, k=4)
                eng.tensor_tensor(
                    out=ey[:].rearrange("p (j q) -> p j q", j=9),
                    in0=y0f[:, 0:64].unsqueeze(1).to_broadcast([128, 9, 64]),
                    in1=y0f[:].rearrange("p (j q) -> p j q", j=9), op=Alu.subtract)
                eng.tensor_tensor(out=eyv, in0=eyv, in1=bc36("cj"), op=Alu.add)

                # single weight tile per axis, taps packed innermost:
                # layout [p, (j*64 + kt)*3 + b]  (b = tap -1/0/+1)
                Wt = wp.tile([128, 3 * 576], f16, tag=f"W_{ax}", name=f"W{ax}")
                wba = Wt[:]
                wm1 = bass.AP(tensor=wba.tensor, offset=wba.offset + 0,
                              ap=[wba.ap[0], [3, 576]])
                w_0 = bass.AP(tensor=wba.tensor, offset=wba.offset + 1,
                              ap=[wba.ap[0], [3, 576]])
                wp1 = bass.AP(tensor=wba.tensor, offset=wba.offset + 2,
                              ap=[wba.ap[0], [3, 576]])
                tmp2 = sp.tile([128, 576], f32, tag=f"tmp2_{ax}", name=f"tmp2{ax}")
                eng.tensor_mul(wm1, w0[:], ey[:])
                eng.tensor_mul(tmp2[:], w1[:], ey[:])
                eng.tensor_sub(w_0, w0[:], wm1)
                eng.tensor_add(w_0, w_0, tmp2[:])
                eng.tensor_sub(wp1, w1[:], tmp2[:])
                W[ax] = Wt

            # gather indices: [128, 64] int32 (fp16-element offsets); GPSIMD
            # arith, final int copy on DVE
            idxf = sp.tile([128, 64], f32, tag="idxf", name="idxf")
            ixv = idxf[:].rearrange("p (k t) -> p k t", k=4)
            nc.vector.tensor_tensor(
                out=ixv, in0=Y0J0["y"][:, 0:64].rearrange("p (k t) -> p k t", k=4),
                in1=bc4("cw4"), op=Alu.mult)
            nc.vector.tensor_add(idxf[:], idxf[:], Y0J0["x"][:, 0:64])
            nc.vector.tensor_tensor(out=ixv, in0=ixv, in1=bc4("cbase4"), op=Alu.add)
            # store as [t, k] so per-tile index slices are contiguous
            nc.vector.tensor_copy(
                out=idx32[:].rearrange("p (t k) -> p t k", k=4).transpose([0, 2, 1]),
                in_=idxf[:].rearrange("p (k t) -> p k t", t=NT))

            # zero row for DRAM pad rows
            zrow = cp.tile([1, VOLW], f16, tag="zrow", name="zrow")
            nc.gpsimd.memset(zrow[:], 0.0)

            f1src = f1[:].rearrange("b (kc cp) i j -> kc cp b (i j)", kc=2)
            f1t2 = {}
            for kc in range(2):
                f1t2[kc] = cp.tile([128, 2048], f16, tag=f"f1_{kc}", name=f"f1t{kc}")

            # matmul N-chunks (512 wide, last 320): grouped into 2048-wide
            # PSUM tiles (4 banks), drained by one ACT copy each.
            # groups: [f2 0:2048), [f2 2048:4096), [pyr -> vol 4096:5440)
            groups = [
                [("f2", n * 512) for n in range(4)],
                [("f2", 2048 + n * 512) for n in range(4)],
                [("pyr", 0), ("pyr", 512), ("pyr", 1024)],
            ]
            gdoff = [0, 2048, 4096]
            gwid = [2048, 2048, 1344]

            def load_f2(b):
                tiles = {}
                for kc in range(2):
                    ft = mp.tile([128, 4096], f16, tag=f"f2_{kc}", name=f"f2t{kc}")
                    pt = mp.tile([128, 1344], f16, tag=f"pyr_{kc}", name=f"pyrt{kc}")
                    fsrc2 = f2[b, kc * 128:(kc + 1) * 128, :, :].rearrange("c u v -> c (u v)")
                    eng = nc.sync if kc == 0 else nc.scalar
                    eng.dma_start(out=ft[:], in_=fsrc2)
                    eng.dma_start(out=pt[:], in_=pyr[b, kc * 128:(kc + 1) * 128, :])
                    tiles[kc] = (ft, pt)
                return tiles

            # b=0 f2 loads first (they gate the first matmuls), then f1
            f2t_next = load_f2(0)
            for kc in range(2):
                eng = nc.sync if kc == 0 else nc.scalar
                eng.dma_start(out=f1t2[kc][:].rearrange("p (b x) -> p b x", b=B),
                              in_=f1src[kc])

            nb = int(os.environ.get("KDBG_NB", str(B)))
            nm_ = int(os.environ.get("KDBG_NM", "4"))

            for b in range(nb):
                f2t = f2t_next
                if b + 1 < nb:
                    f2t_next = load_f2(b + 1)

                for m in range(nm_):
                    t = b * 4 + m
                    volsb = fp.tile([128, VOLW], f16, tag="volsb", name="volsb", bufs=3)
                    for gi, grp in enumerate(groups):
                        ps = pp.tile([128, 2048], f32, tag="ps", name="ps")
                        # kc outer: consecutive matmuls share the stationary lhsT
                        for kc in range(2):
                            for pi, (srcname, soff) in enumerate(grp):
                                width = 320 if (srcname == "pyr" and soff == 1024) else 512
                                rhs_t = f2t[kc][0] if srcname == "f2" else f2t[kc][1]
                                nc.tensor.matmul(
                                    out=ps[:, pi * 512:pi * 512 + width],
                                    lhsT=f1t2[kc][:, b * 512 + m * 128:b * 512 + (m + 1) * 128],
                                    rhs=rhs_t[:, soff:soff + width],
                                    start=(kc == 0), stop=(kc == 1))
                        nc.scalar.copy(out=volsb[:, gdoff[gi]:gdoff[gi] + gwid[gi]],
                                       in_=ps[:, 0:gwid[gi]])

                    vols = dp.tile([130, VOLW], f16, tag="vols", name="vols")
                    nc.scalar.dma_start(out=vols[0:1, :], in_=zrow[:])
                    nc.scalar.dma_start(out=vols[129:130, :], in_=zrow[:])
                    nc.scalar.dma_start(out=vols[1:129, :], in_=volsb[:])

                    vflat = vols[:].rearrange("a b -> (a b)").unsqueeze(1)
                    Fk = []
                    for k in range(4):
                        flen = 10 * HK[k] + 11
                        ftile = fp.tile([128, flen], f16, tag=f"F{k}", name=f"F{k}")
                        nc.gpsimd.indirect_dma_start(
                            out=ftile[:], out_offset=None, in_=vflat,
                            in_offset=bass.IndirectOffsetOnAxis(
                                ap=idx32[:, t * 4 + k:t * 4 + k + 1], axis=0))
                        Fk.append(ftile)

                    # ---- separable 3-tap reconstruction (all fp16/packed) ----
                    rect4 = fp.tile([128, 324], f16, tag="rect4", name="rect4")
                    for k in range(4):
                        w_k = HK[k]
                        fap = Fk[k][:]
                        kt3 = (k * NT + t) * 3
                        # stage X over dims (x, row, tap):
                        #   tmpx[x,row,b] = F[row*w + x + b] * WX[b, j=x]
                        tmpx = fp.tile([128, 297], f16, tag="tmpx", name="tmpx")
                        fin = bass.AP(tensor=fap.tensor, offset=fap.offset,
                                      ap=[fap.ap[0], [1, 9], [w_k, 11], [1, 3]])
                        wxv = W["x"][:]
                        wxap = bass.AP(tensor=wxv.tensor, offset=wxv.offset + kt3,
                                       ap=[wxv.ap[0], [192, 9], [0, 11], [1, 3]])
                        txv = bass.AP(tensor=tmpx[:].tensor, offset=tmpx[:].offset,
                                      ap=[tmpx[:].ap[0], [33, 9], [3, 11], [1, 3]])
                        nc.vector.tensor_tensor(out=txv, in0=fin, in1=wxap, op=Alu.mult)
                        # G laid out x-major: G[x*12 + row]
                        G = fp.tile([128, 108], f16, tag="G", name="G")
                        gv = G[:]
                        with nc.allow_low_precision(reason="3-tap fp16 sum"):
                            nc.vector.tensor_reduce(
                                out=bass.AP(tensor=gv.tensor, offset=gv.offset,
                                            ap=[gv.ap[0], [12, 9], [1, 11]]),
                                in_=txv,
                                axis=mybir.AxisListType.X, op=Alu.add)
                        # stage Y over dims (i, j, a):
                        #   tmpy[i,j,a] = G[i*12 + j + a] * WY[a, j]
                        tmpy = fp.tile([128, 243], f16, tag="tmpy", name="tmpy")
                        gin = bass.AP(tensor=gv.tensor, offset=gv.offset,
                                      ap=[gv.ap[0], [12, 9], [1, 9], [1, 3]])
                        wyv = W["y"][:]
                        wyap = bass.AP(tensor=wyv.tensor, offset=wyv.offset + kt3,
                                       ap=[wyv.ap[0], [0, 9], [192, 9], [1, 3]])
                        tyv = bass.AP(tensor=tmpy[:].tensor, offset=tmpy[:].offset,
                                      ap=[tmpy[:].ap[0], [27, 9], [3, 9], [1, 3]])
                        nc.vector.tensor_tensor(out=tyv, in0=gin, in1=wyap, op=Alu.mult)
                        # rect layout per level: rect[k*81 + x*9 + y]
                        rv = rect4[:]
                        with nc.allow_low_precision(reason="3-tap fp16 sum"):
                            nc.vector.tensor_reduce(
                                out=bass.AP(tensor=rv.tensor, offset=rv.offset + k * 81,
                                            ap=[rv.ap[0], [9, 9], [1, 9]]),
                                in_=tyv,
                                axis=mybir.AxisListType.X, op=Alu.add)

                    nc.sync.dma_start(out=out[t * 128:(t + 1) * 128, :], in_=rect4[:])

    nc.compile()
    return nc


def _get_nc():
    if "nc" not in _CACHE:
        _CACHE["nc"] = _build()
    return _CACHE["nc"]


def _pool_pyr(feat2):
    """Host-side sum-pooled pyramids of f2 (levels 1-3), fp16, [B,C,1344]."""
    l1 = feat2.reshape(B, C, 32, 2, 32, 2).sum(axis=(3, 5))
    l2 = l1.reshape(B, C, 16, 2, 16, 2).sum(axis=(3, 5))
    l3 = l2.reshape(B, C, 8, 2, 8, 2).sum(axis=(3, 5))
    return np.concatenate(
        [l1.reshape(B, C, 1024), l2.reshape(B, C, 256), l3.reshape(B, C, 64)],
        axis=2).astype(np.float16)


def _in_maps(feat1, feat2, curr_flow):
    f2h = np.ascontiguousarray(feat2, dtype=np.float16)
    pyrh = _pool_pyr(np.asarray(feat2, dtype=np.float32))
    zpad = np.zeros((1, VOLW), dtype=np.float16)
    maps = []
    for core in range(NCORES):
        m = dict(_consts(core))
        sl = slice(8 * core, 8 * core + 8)
        m["f1"] = np.ascontiguousarray(feat1[:, :, sl, :], dtype=np.float16)
        m["f2"] = f2h
        m["pyr"] = pyrh
        m["flow"] = np.ascontiguousarray(curr_flow[:, :, sl, :], dtype=np.float32)
        maps.append(m)
    return maps


def _assemble(outs):
    # per core: [2048, 324] fp16 rect grids -> diamond-extract on host
    # -> [4, 8, 64, 4, 41]; concat cores on i axis
    parts = []
    for o in outs:
        r = np.asarray(o, dtype=np.float32).reshape(B, RI, W8, 4, 81)
        parts.append(r[..., IDX81])
    return np.concatenate(parts, axis=1)


def kernel(feat1, feat2, curr_flow):
    from concourse.bass_utils import run_bass_kernel_spmd

    nc = _get_nc()
    res = run_bass_kernel_spmd(nc, _in_maps(feat1, feat2, curr_flow), list(range(NCORES)))
    return _assemble([np.asarray(res.results[i]["out"]) for i in range(NCORES)])

